# revision 13
# baseline (speedup 1.0000x reference)
"""Trainium2 Bass kernel for nn_CNN_PHMM_VAE loss (profile-HMM forward + VAE KLD).

Data parallel over 8 NeuronCores (64 examples per core). PHMM forward in
probability space with periodic rescaling. Emission lookups are shipped as
host-gathered bf16 tables:
  ee [b,l,k] = A1[b,k+1]*exp(emission[b,k,s[b,l]])      (mu-state update)
  eeU[b,l,j] = U[b,j+2]*ee[b,l,j]                       (beta fold: scan input)

Per step, 7 DVE ops ordered [scan, r12, t, ya, mu, muU, y'] so every
producer->consumer pair is >=2 apart (no adjacent-dependency stalls):
  delta = scan(V, MUU)        ; r12 = [G1-G2|G2] o X
  t     = y + delta           ; ya  = r12.lo + r12.hi
  mu'   = ee_l o t            ; MUU[2:] = eeU_l o t
  y'    = mu' + ya
The beta-mul of the baseline is folded into MUU via eeU (MUU[k] = beta[k]);
MUU[0:2] are sig-dependent constants refreshed at rescale.
"""
import numpy as np

B, L, K, E = 512, 256, 64, 16
K1 = K + 1
N_CORES = 8
BPC = B // N_CORES
R = 64
LOGACC0 = -60.0
NEG = -100.0
M2M, M2I, M2D, I2M, I2I, D2M, D2D = 0, 1, 2, 3, 4, 5, 6

# --- small f32 table layout -------------------------------------------------
OFF_X0 = 0             # 132: initial [mu | pad | y | pad]
OFF_V = 132            # 65
OFF_GG = 198           # 132: [G1-G2 | G2] aligned to X layout
OFF_MUU0 = 330         # 65: initial MUU (= beta column at l=0)
OFF_A3C0 = 396
OFF_U1A1C0 = 397       # U[1]*A1[0]  (must stay adjacent after A3C0)
OFF_A1C0 = 398
OFF_SIG0 = 399
OFF_MUS = 400          # 16
OFF_LV = 416           # 16
TBL_W = 432

XW = 132   # state: mu 0..64, pad, y 66..130, pad
YO = 66

_CACHED = {}


def _host_tables(batch_input, transition_probs, emission_probs, mus, logvars):
    import ml_dtypes

    a = np.asarray(transition_probs, np.float64)
    Earr = np.exp(np.asarray(emission_probs, np.float64))
    s = np.asarray(batch_input)
    A1 = np.exp(a[:, :, M2M])
    A2 = np.exp(a[:, :, I2M])
    A3 = np.exp(a[:, :, D2M])
    B1 = 0.25 * np.exp(a[:, :, M2I])
    B2 = 0.25 * np.exp(a[:, :, I2I])
    C1 = np.exp(a[:, :, M2D])
    C2 = np.exp(a[:, :, D2D])

    U = np.zeros((B, K1)); V = np.zeros((B, K1))
    U[:, 1:] = A3[:, 1:] * C1[:, :-1] / A1[:, :-1]
    V[:, 1:] = A3[:, 1:] * C2[:, :-1] / A3[:, :-1]
    G1 = A2 * B1 / A1
    G2 = B2

    Etil = A1[:, 1:, None] * Earr
    ee = Etil[np.arange(B)[:, None, None], np.arange(K)[None, None, :],
              s[:, :, None]]                      # (B, L, K)
    ee_bf = np.asarray(ee, ml_dtypes.bfloat16).reshape(B, L * K)
    eeU = np.zeros((B, L, K))
    eeU[:, :, :K - 1] = U[:, None, 2:] * ee[:, :, :K - 1]
    eeU_bf = np.asarray(eeU, ml_dtypes.bfloat16).reshape(B, L * K)

    sig0 = np.exp(NEG - LOGACC0)
    e0 = np.exp(-LOGACC0)
    mu0 = np.empty((B, K1)); iot0 = np.empty((B, K1))
    mu0[:, 0] = A1[:, 0] * e0
    mu0[:, 1:] = A1[:, 1:] * sig0
    iot0[:, :] = A2 * sig0
    muu0 = np.empty((B, K1))
    muu0[:, 0] = A3[:, 0] * sig0
    muu0[:, 1:] = U[:, 1:] * mu0[:, :-1]

    tbl = np.zeros((B, TBL_W), np.float32)
    tbl[:, OFF_X0:OFF_X0 + K1] = mu0
    tbl[:, OFF_X0 + YO:OFF_X0 + YO + K1] = mu0 + iot0
    tbl[:, OFF_V:OFF_V + K1] = V
    tbl[:, OFF_GG:OFF_GG + K1] = G1 - G2
    tbl[:, OFF_GG + YO:OFF_GG + YO + K1] = G2
    tbl[:, OFF_MUU0:OFF_MUU0 + K1] = muu0
    tbl[:, OFF_A1C0] = A1[:, 0]
    tbl[:, OFF_A3C0] = A3[:, 0]
    tbl[:, OFF_SIG0] = sig0
    tbl[:, OFF_U1A1C0] = U[:, 1] * A1[:, 0]
    tbl[:, OFF_MUS:OFF_MUS + E] = np.asarray(mus, np.float32)
    tbl[:, OFF_LV:OFF_LV + E] = np.asarray(logvars, np.float32)
    return tbl, ee_bf, eeU_bf


def _build_bass():
    import concourse.tile as tile
    from concourse import bacc, mybir
    from contextlib import ExitStack

    f32 = mybir.dt.float32
    bf = mybir.dt.bfloat16
    mult = mybir.AluOpType.mult
    add = mybir.AluOpType.add
    mx_op = mybir.AluOpType.max
    AF = mybir.ActivationFunctionType

    nc = bacc.Bacc("TRN2", target_bir_lowering=False, debug=False,
                   num_devices=N_CORES)
    tbl_d = nc.dram_tensor("tbl", [BPC, TBL_W], f32, kind="ExternalInput").ap()
    ee_d = nc.dram_tensor("ee", [BPC, L * K], bf, kind="ExternalInput").ap()
    eeU_d = nc.dram_tensor("eeU", [BPC, L * K], bf, kind="ExternalInput").ap()
    out_d = nc.dram_tensor("loss", [BPC, 1], f32, kind="ExternalOutput").ap()

    with tile.TileContext(nc) as tc, ExitStack() as ctx:
        ctx.enter_context(nc.allow_low_precision(
            reason="bf16 DP state validated to ~2e-4 per-example on the loss"))
        pool = ctx.enter_context(tc.tile_pool(name="p", bufs=1))

        TBL = pool.tile([BPC, TBL_W], f32, tag="TBL", name="TBL")
        EEt = pool.tile([BPC, L * K], bf, tag="EE", name="EE")
        EUt = pool.tile([BPC, L * K], bf, tag="EU", name="EU")
        nc.scalar.dma_start(TBL[:, :], tbl_d[:, :])
        CW = L * K // 4
        bounds = [0, CW // 4, CW, 2 * CW, 3 * CW, 4 * CW]
        for c in range(len(bounds) - 1):
            lo, hi = bounds[c], bounds[c + 1]
            nc.scalar.dma_start(EEt[:, lo:hi], ee_d[:, lo:hi])
            nc.gpsimd.dma_start(EUt[:, lo:hi], eeU_d[:, lo:hi])

        def tb(off, n):
            return TBL[:, off:off + n]

        v = nc.vector

        # KLD
        ev = pool.tile([BPC, E], f32, tag="ev", name="ev")
        sq = pool.tile([BPC, E], f32, tag="sq", name="sq")
        w1 = pool.tile([BPC, E], f32, tag="w1", name="w1")
        w2 = pool.tile([BPC, E], f32, tag="w2", name="w2")
        red = pool.tile([BPC, 1], f32, tag="red", name="red")
        kld = pool.tile([BPC, 1], f32, tag="kld", name="kld")
        nc.scalar.activation(ev[:, :], tb(OFF_LV, E), AF.Exp)
        nc.scalar.activation(sq[:, :], tb(OFF_MUS, E), AF.Square)
        v.tensor_sub(w1[:, :], tb(OFF_LV, E), sq[:, :])
        v.tensor_sub(w2[:, :], w1[:, :], ev[:, :])
        v.tensor_reduce(red[:, :], w2[:, :], mybir.AxisListType.X, add)
        v.tensor_scalar(kld[:, :], red[:, :], -0.5, -float(E) / 2.0, mult, add)

        # ---- DP loop state --------------------------------------------------
        x_pp = [pool.tile([BPC, XW], bf, tag="x_a", name="x_a"),
                pool.tile([BPC, XW], bf, tag="x_b", name="x_b")]
        MUU = pool.tile([BPC, K1], bf, tag="MUU", name="MUU")
        Vb = pool.tile([BPC, K1], bf, tag="Vb", name="Vb")
        GGb = pool.tile([BPC, XW], bf, tag="GGb", name="GGb")
        delta = pool.tile([BPC, K1], bf, tag="delta", name="delta")
        t = pool.tile([BPC, K], bf, tag="t", name="t")
        r12 = pool.tile([BPC, XW], bf, tag="r12", name="r12")
        ya = pool.tile([BPC, K1 + 1], bf, tag="ya", name="ya")
        sig = pool.tile([BPC, 1], f32, tag="sig", name="sig")
        rmxb = pool.tile([BPC, 1], bf, tag="rmxb", name="rmxb")
        mxt = pool.tile([BPC, 1], f32, tag="mxt", name="mxt")
        NRS = len([l for l in range(L) if (l + 1) % R == 0 and (l + 1) < L])
        rhist = pool.tile([BPC, NRS], f32, tag="rhist", name="rhist")

        v.memset(x_pp[0][:, :], 0.0)
        v.memset(x_pp[1][:, :], 0.0)
        v.memset(r12[:, :], 0.0)
        v.tensor_copy(Vb[:, :], tb(OFF_V, K1))
        v.memset(GGb[:, :], 0.0)
        v.tensor_copy(GGb[:, 0:K1], tb(OFF_GG, K1))
        v.tensor_copy(GGb[:, YO:YO + K1], tb(OFF_GG + YO, K1))
        v.tensor_copy(x_pp[0][:, 0:K1], tb(OFF_X0, K1))
        v.tensor_copy(x_pp[0][:, YO:YO + K1], tb(OFF_X0 + YO, K1))
        v.tensor_copy(MUU[:, :], tb(OFF_MUU0, K1))
        v.tensor_copy(sig[:, :], tb(OFF_SIG0, 1))
        v.tensor_mul(x_pp[1][:, 0:1], tb(OFF_A1C0, 1), sig[:, :])

        g = nc.gpsimd

        def dp_step(l):
            X, Xn = x_pp[l % 2], x_pp[(l + 1) % 2]
            v.tensor_tensor_scan(delta[:, :], Vb[:, :], MUU[:, :], 0.0,
                                 mult, add)
            v.tensor_mul(r12[:, :], GGb[:, :], X[:, :])
            v.tensor_add(t[:, :], X[:, YO:YO + K], delta[:, 0:K])
            g.tensor_add(ya[:, :], r12[:, 0:K1 + 1], r12[:, YO:YO + K1 + 1])
            v.tensor_mul(Xn[:, 1:K1], EEt[:, l * K:(l + 1) * K], t[:, :])
            v.tensor_mul(MUU[:, 2:K1], EUt[:, l * K:l * K + K - 1],
                         t[:, 0:K - 1])
            v.tensor_add(Xn[:, YO:YO + K1 + 1], Xn[:, 0:K1 + 1], ya[:, :])

        def rescale(i, l):
            cur = (l + 1) % 2
            Xc = x_pp[cur]
            x_stale = x_pp[1 - cur]
            rmx = rhist[:, i:i + 1]
            v.tensor_reduce(mxt[:, :], Xc[:, YO:YO + K1], mybir.AxisListType.X,
                            mx_op)
            v.reciprocal(rmxb[:, :], mxt[:, :])
            v.tensor_copy(rmx, rmxb[:, :])
            v.tensor_scalar_mul(Xc[:, :], Xc[:, :], rmx)
            v.tensor_scalar_mul(sig[:, :], sig[:, :], rmx)
            v.tensor_scalar_mul(MUU[:, 0:2], tb(OFF_A3C0, 2), sig[:, :])
            v.tensor_mul(x_stale[:, 0:1], tb(OFF_A1C0, 1), sig[:, :])

        for l in range(L):
            dp_step(l)
            if l == 0:
                v.tensor_mul(x_pp[0][:, 0:1], tb(OFF_A1C0, 1), sig[:, :])
                v.tensor_mul(MUU[:, 1:2], tb(OFF_U1A1C0, 1), sig[:, :])
            if (l + 1) % R == 0 and (l + 1) < L:
                rescale((l + 1) // R - 1, l)

        Xf = x_pp[L % 2]
        tf = pool.tile([BPC, K1], f32, tag="tf", name="tf")
        lnp = pool.tile([BPC, 1], f32, tag="lnp", name="lnp")
        lnr = pool.tile([BPC, NRS], f32, tag="lnr", name="lnr")
        sumlr = pool.tile([BPC, 1], f32, tag="sumlr", name="sumlr")
        lacc = pool.tile([BPC, 1], f32, tag="lacc", name="lacc")
        nv = pool.tile([BPC, 1], f32, tag="nv", name="nv")
        v.tensor_tensor_scan(delta[:, :], Vb[:, :], MUU[:, :], 0.0, mult, add)
        v.tensor_add(tf[:, :], Xf[:, YO:YO + K1], delta[:, :])
        nc.scalar.activation(lnp[:, :], tf[:, K:K1], AF.Ln)
        nc.scalar.activation(lnr[:, :], rhist[:, :], AF.Ln)
        v.tensor_reduce(sumlr[:, :], lnr[:, :], mybir.AxisListType.X, add)
        v.tensor_scalar(lacc[:, :], sumlr[:, :], -1.0, LOGACC0, mult, add)
        v.tensor_add(nv[:, :], lnp[:, :], lacc[:, :])
        loss_t = pool.tile([BPC, 1], f32, tag="loss_t", name="loss_t")
        v.tensor_sub(loss_t[:, :], kld[:, :], nv[:, :])
        nc.sync.dma_start(out_d[:, :], loss_t[:, :])

    nc.compile()
    return nc


def _get_nc():
    if "nc" not in _CACHED:
        _CACHED["nc"] = _build_bass()
    return _CACHED["nc"]


def kernel(batch_input, transition_probs, emission_probs, mus, logvars):
    from concourse.bass_utils import run_bass_kernel_spmd

    tbl, ee, eeU = _host_tables(batch_input, transition_probs, emission_probs,
                                mus, logvars)
    nc = _get_nc()
    in_maps = [{"tbl": tbl[c * BPC:(c + 1) * BPC],
                "ee": ee[c * BPC:(c + 1) * BPC],
                "eeU": eeU[c * BPC:(c + 1) * BPC]} for c in range(N_CORES)]
    res = run_bass_kernel_spmd(nc, in_maps, list(range(N_CORES)))
    losses = np.concatenate([np.asarray(r["loss"])[:, 0] for r in res.results])
    return np.float32(np.mean(losses.astype(np.float64)))


# revision 14
# speedup vs baseline: 1.1693x; 1.1693x over previous
"""Trainium2 Bass kernel for nn_CNN_PHMM_VAE loss (profile-HMM forward + VAE KLD).

Data parallel over 8 NeuronCores (64 examples per core). PHMM forward in
probability space with periodic rescaling. Emission lookups are shipped as
host-gathered bf16 tables:
  ee [b,l,k] = A1[b,k+1]*exp(emission[b,k,s[b,l]])      (mu-state update)
  eeU[b,l,j] = U[b,j+2]*ee[b,l,j]                       (beta fold: scan input)

Per step, 7 DVE ops ordered [scan, r12, t, ya, mu, muU, y'] so every
producer->consumer pair is >=2 apart (no adjacent-dependency stalls):
  delta = scan(V, MUU)        ; r12 = [G1-G2|G2] o X
  t     = y + delta           ; ya  = r12.lo + r12.hi
  mu'   = ee_l o t            ; MUU[2:] = eeU_l o t
  y'    = mu' + ya
The beta-mul of the baseline is folded into MUU via eeU (MUU[k] = beta[k]);
MUU[0:2] are sig-dependent constants refreshed at rescale.
"""
import numpy as np

B, L, K, E = 512, 256, 64, 16
K1 = K + 1
N_CORES = 8
BPC = B // N_CORES
R = 64
LOGACC0 = -60.0
NEG = -100.0
M2M, M2I, M2D, I2M, I2I, D2M, D2D = 0, 1, 2, 3, 4, 5, 6

# --- small f32 table layout -------------------------------------------------
OFF_X0 = 0             # 132: initial [mu | pad | y | pad]
OFF_V = 132            # 65
OFF_GG = 198           # 132: [G1-G2 | G2] aligned to X layout
OFF_MUU0 = 330         # 65: initial MUU (= beta column at l=0)
OFF_A3C0 = 396
OFF_U1A1C0 = 397       # U[1]*A1[0]  (must stay adjacent after A3C0)
OFF_A1C0 = 398
OFF_SIG0 = 399
OFF_MUS = 400          # 16
OFF_LV = 416           # 16
TBL_W = 432

XW = 132   # state: mu 0..64, pad, y 66..130, pad
YO = 66

_CACHED = {}


def _host_tables(batch_input, transition_probs, emission_probs, mus, logvars):
    import ml_dtypes

    a = np.asarray(transition_probs, np.float64)
    Earr = np.exp(np.asarray(emission_probs, np.float64))
    s = np.asarray(batch_input)
    A1 = np.exp(a[:, :, M2M])
    A2 = np.exp(a[:, :, I2M])
    A3 = np.exp(a[:, :, D2M])
    B1 = 0.25 * np.exp(a[:, :, M2I])
    B2 = 0.25 * np.exp(a[:, :, I2I])
    C1 = np.exp(a[:, :, M2D])
    C2 = np.exp(a[:, :, D2D])

    U = np.zeros((B, K1)); V = np.zeros((B, K1))
    U[:, 1:] = A3[:, 1:] * C1[:, :-1] / A1[:, :-1]
    V[:, 1:] = A3[:, 1:] * C2[:, :-1] / A3[:, :-1]
    G1 = A2 * B1 / A1
    G2 = B2

    Etil = A1[:, 1:, None] * Earr
    ee = Etil[np.arange(B)[:, None, None], np.arange(K)[None, None, :],
              s[:, :, None]]                      # (B, L, K)
    ee_bf = np.asarray(ee, ml_dtypes.bfloat16).reshape(B, L * K)
    eeU = np.zeros((B, L, K))
    eeU[:, :, :K - 1] = U[:, None, 2:] * ee[:, :, :K - 1]
    eeU_bf = np.asarray(eeU, ml_dtypes.bfloat16).reshape(B, L * K)

    sig0 = np.exp(NEG - LOGACC0)
    e0 = np.exp(-LOGACC0)
    mu0 = np.empty((B, K1)); iot0 = np.empty((B, K1))
    mu0[:, 0] = A1[:, 0] * e0
    mu0[:, 1:] = A1[:, 1:] * sig0
    iot0[:, :] = A2 * sig0
    muu0 = np.empty((B, K1))
    muu0[:, 0] = A3[:, 0] * sig0
    muu0[:, 1:] = U[:, 1:] * mu0[:, :-1]

    tbl = np.zeros((B, TBL_W), np.float32)
    tbl[:, OFF_X0:OFF_X0 + K1] = mu0
    tbl[:, OFF_X0 + YO:OFF_X0 + YO + K1] = mu0 + iot0
    tbl[:, OFF_V:OFF_V + K1] = V
    tbl[:, OFF_GG:OFF_GG + K1] = G1 - G2
    tbl[:, OFF_GG + YO:OFF_GG + YO + K1] = G2
    tbl[:, OFF_MUU0:OFF_MUU0 + K1] = muu0
    tbl[:, OFF_A1C0] = A1[:, 0]
    tbl[:, OFF_A3C0] = A3[:, 0]
    tbl[:, OFF_SIG0] = sig0
    tbl[:, OFF_U1A1C0] = U[:, 1] * A1[:, 0]
    tbl[:, OFF_MUS:OFF_MUS + E] = np.asarray(mus, np.float32)
    tbl[:, OFF_LV:OFF_LV + E] = np.asarray(logvars, np.float32)
    return tbl, ee_bf, eeU_bf


def _build_bass():
    import concourse.tile as tile
    from concourse import bacc, mybir
    from contextlib import ExitStack

    f32 = mybir.dt.float32
    bf = mybir.dt.bfloat16
    mult = mybir.AluOpType.mult
    add = mybir.AluOpType.add
    mx_op = mybir.AluOpType.max
    AF = mybir.ActivationFunctionType

    nc = bacc.Bacc("TRN2", target_bir_lowering=False, debug=False,
                   num_devices=N_CORES)
    tbl_d = nc.dram_tensor("tbl", [BPC, TBL_W], f32, kind="ExternalInput").ap()
    ee_d = nc.dram_tensor("ee", [BPC, L * K], bf, kind="ExternalInput").ap()
    eeU_d = nc.dram_tensor("eeU", [BPC, L * K], bf, kind="ExternalInput").ap()
    out_d = nc.dram_tensor("loss", [BPC, 1], f32, kind="ExternalOutput").ap()

    with tile.TileContext(nc) as tc, ExitStack() as ctx:
        ctx.enter_context(nc.allow_low_precision(
            reason="bf16 DP state validated to ~2e-4 per-example on the loss"))
        pool = ctx.enter_context(tc.tile_pool(name="p", bufs=1))

        TBL = pool.tile([BPC, TBL_W], f32, tag="TBL", name="TBL")
        EEt = pool.tile([BPC, L * K], bf, tag="EE", name="EE")
        EUt = pool.tile([BPC, L * K], bf, tag="EU", name="EU")
        nc.scalar.dma_start(TBL[:, :], tbl_d[:, :])
        CW = L * K // 4
        bounds = [0, CW // 4, CW, 2 * CW, 3 * CW, 4 * CW]
        for c in range(len(bounds) - 1):
            lo, hi = bounds[c], bounds[c + 1]
            nc.scalar.dma_start(EEt[:, lo:hi], ee_d[:, lo:hi])
            nc.gpsimd.dma_start(EUt[:, lo:hi], eeU_d[:, lo:hi])

        def tb(off, n):
            return TBL[:, off:off + n]

        v = nc.vector

        # KLD
        ev = pool.tile([BPC, E], f32, tag="ev", name="ev")
        sq = pool.tile([BPC, E], f32, tag="sq", name="sq")
        w1 = pool.tile([BPC, E], f32, tag="w1", name="w1")
        w2 = pool.tile([BPC, E], f32, tag="w2", name="w2")
        red = pool.tile([BPC, 1], f32, tag="red", name="red")
        kld = pool.tile([BPC, 1], f32, tag="kld", name="kld")
        nc.scalar.activation(ev[:, :], tb(OFF_LV, E), AF.Exp)
        nc.scalar.activation(sq[:, :], tb(OFF_MUS, E), AF.Square)
        v.tensor_sub(w1[:, :], tb(OFF_LV, E), sq[:, :])
        v.tensor_sub(w2[:, :], w1[:, :], ev[:, :])
        v.tensor_reduce(red[:, :], w2[:, :], mybir.AxisListType.X, add)
        v.tensor_scalar(kld[:, :], red[:, :], -0.5, -float(E) / 2.0, mult, add)

        # ---- DP loop state --------------------------------------------------
        x_pp = [pool.tile([BPC, XW], bf, tag="x_a", name="x_a"),
                pool.tile([BPC, XW], bf, tag="x_b", name="x_b")]
        MUU = pool.tile([BPC, K1], bf, tag="MUU", name="MUU")
        Vb = pool.tile([BPC, K1], bf, tag="Vb", name="Vb")
        GGb = pool.tile([BPC, XW], bf, tag="GGb", name="GGb")
        delta = pool.tile([BPC, K1], bf, tag="delta", name="delta")
        t = pool.tile([BPC, K], bf, tag="t", name="t")
        r12 = pool.tile([BPC, XW], bf, tag="r12", name="r12")
        ya = pool.tile([BPC, K1 + 1], bf, tag="ya", name="ya")
        sig = pool.tile([BPC, 1], f32, tag="sig", name="sig")
        rmxb = pool.tile([BPC, 1], bf, tag="rmxb", name="rmxb")
        mxt = pool.tile([BPC, 1], f32, tag="mxt", name="mxt")
        NRS = len([l for l in range(L) if (l + 1) % R == 0 and (l + 1) < L])
        rhist = pool.tile([BPC, NRS], f32, tag="rhist", name="rhist")

        v.memset(x_pp[0][:, :], 0.0)
        v.memset(x_pp[1][:, :], 0.0)
        v.memset(r12[:, :], 0.0)
        v.tensor_copy(Vb[:, :], tb(OFF_V, K1))
        v.memset(GGb[:, :], 0.0)
        v.tensor_copy(GGb[:, 0:K1], tb(OFF_GG, K1))
        v.tensor_copy(GGb[:, YO:YO + K1], tb(OFF_GG + YO, K1))
        v.tensor_copy(x_pp[0][:, 0:K1], tb(OFF_X0, K1))
        v.tensor_copy(x_pp[0][:, YO:YO + K1], tb(OFF_X0 + YO, K1))
        v.tensor_copy(MUU[:, :], tb(OFF_MUU0, K1))
        v.tensor_copy(sig[:, :], tb(OFF_SIG0, 1))
        v.tensor_mul(x_pp[1][:, 0:1], tb(OFF_A1C0, 1), sig[:, :])

        g = nc.gpsimd

        def dp_step(l):
            X, Xn = x_pp[l % 2], x_pp[(l + 1) % 2]
            v.tensor_tensor_scan(delta[:, :], Vb[:, :], MUU[:, :], 0.0,
                                 mult, add)
            v.tensor_mul(r12[:, :], GGb[:, :], X[:, :])
            v.tensor_add(t[:, :], X[:, YO:YO + K], delta[:, 0:K])
            v.tensor_add(ya[:, :], r12[:, 0:K1 + 1], r12[:, YO:YO + K1 + 1])
            v.tensor_mul(Xn[:, 1:K1], EEt[:, l * K:(l + 1) * K], t[:, :])
            v.tensor_mul(MUU[:, 2:K1], EUt[:, l * K:l * K + K - 1],
                         t[:, 0:K - 1])
            v.tensor_add(Xn[:, YO:YO + K1 + 1], Xn[:, 0:K1 + 1], ya[:, :])

        def rescale(i, l):
            cur = (l + 1) % 2
            Xc = x_pp[cur]
            x_stale = x_pp[1 - cur]
            rmx = rhist[:, i:i + 1]
            v.tensor_reduce(mxt[:, :], Xc[:, YO:YO + K1], mybir.AxisListType.X,
                            mx_op)
            v.reciprocal(rmxb[:, :], mxt[:, :])
            v.tensor_copy(rmx, rmxb[:, :])
            v.tensor_scalar_mul(Xc[:, :], Xc[:, :], rmx)
            v.tensor_scalar_mul(sig[:, :], sig[:, :], rmx)
            v.tensor_scalar_mul(MUU[:, 0:2], tb(OFF_A3C0, 2), sig[:, :])
            v.tensor_mul(x_stale[:, 0:1], tb(OFF_A1C0, 1), sig[:, :])

        for l in range(L):
            dp_step(l)
            if l == 0:
                v.tensor_mul(x_pp[0][:, 0:1], tb(OFF_A1C0, 1), sig[:, :])
                v.tensor_mul(MUU[:, 1:2], tb(OFF_U1A1C0, 1), sig[:, :])
            if (l + 1) % R == 0 and (l + 1) < L:
                rescale((l + 1) // R - 1, l)

        Xf = x_pp[L % 2]
        tf = pool.tile([BPC, K1], f32, tag="tf", name="tf")
        lnp = pool.tile([BPC, 1], f32, tag="lnp", name="lnp")
        lnr = pool.tile([BPC, NRS], f32, tag="lnr", name="lnr")
        sumlr = pool.tile([BPC, 1], f32, tag="sumlr", name="sumlr")
        lacc = pool.tile([BPC, 1], f32, tag="lacc", name="lacc")
        nv = pool.tile([BPC, 1], f32, tag="nv", name="nv")
        v.tensor_tensor_scan(delta[:, :], Vb[:, :], MUU[:, :], 0.0, mult, add)
        v.tensor_add(tf[:, :], Xf[:, YO:YO + K1], delta[:, :])
        nc.scalar.activation(lnp[:, :], tf[:, K:K1], AF.Ln)
        nc.scalar.activation(lnr[:, :], rhist[:, :], AF.Ln)
        v.tensor_reduce(sumlr[:, :], lnr[:, :], mybir.AxisListType.X, add)
        v.tensor_scalar(lacc[:, :], sumlr[:, :], -1.0, LOGACC0, mult, add)
        v.tensor_add(nv[:, :], lnp[:, :], lacc[:, :])
        loss_t = pool.tile([BPC, 1], f32, tag="loss_t", name="loss_t")
        v.tensor_sub(loss_t[:, :], kld[:, :], nv[:, :])
        nc.sync.dma_start(out_d[:, :], loss_t[:, :])

    nc.compile()
    return nc


def _get_nc():
    if "nc" not in _CACHED:
        _CACHED["nc"] = _build_bass()
    return _CACHED["nc"]


def kernel(batch_input, transition_probs, emission_probs, mus, logvars):
    from concourse.bass_utils import run_bass_kernel_spmd

    tbl, ee, eeU = _host_tables(batch_input, transition_probs, emission_probs,
                                mus, logvars)
    nc = _get_nc()
    in_maps = [{"tbl": tbl[c * BPC:(c + 1) * BPC],
                "ee": ee[c * BPC:(c + 1) * BPC],
                "eeU": eeU[c * BPC:(c + 1) * BPC]} for c in range(N_CORES)]
    res = run_bass_kernel_spmd(nc, in_maps, list(range(N_CORES)))
    losses = np.concatenate([np.asarray(r["loss"])[:, 0] for r in res.results])
    return np.float32(np.mean(losses.astype(np.float64)))


# revision 16
# speedup vs baseline: 1.4001x; 1.1974x over previous
"""Trainium2 Bass kernel for nn_CNN_PHMM_VAE loss (profile-HMM forward + VAE KLD).

Data parallel over 8 NeuronCores (64 examples per core). PHMM forward in
probability space with periodic rescaling. Emission lookups are shipped as
host-gathered bf16 tables:
  ee [b,l,k] = A1[b,k+1]*exp(emission[b,k,s[b,l]])      (mu-state update)
  eeU[b,l,j] = U[b,j+2]*ee[b,l,j]                       (beta fold: scan input)

Per step, 7 DVE ops ordered [scan, r12, t, ya, mu, muU, y'] so every
producer->consumer pair is >=2 apart (no adjacent-dependency stalls):
  delta = scan(V, MUU)        ; r12 = [G1-G2|G2] o X
  t     = y + delta           ; ya  = r12.lo + r12.hi
  mu'   = ee_l o t            ; MUU[2:] = eeU_l o t
  y'    = mu' + ya
The beta-mul of the baseline is folded into MUU via eeU (MUU[k] = beta[k]);
MUU[0:2] are sig-dependent constants refreshed at rescale.
"""
import numpy as np

B, L, K, E = 512, 256, 64, 16
K1 = K + 1
N_CORES = 8
BPC = B // N_CORES
R = 64
LOGACC0 = -60.0
NEG = -100.0
M2M, M2I, M2D, I2M, I2I, D2M, D2D = 0, 1, 2, 3, 4, 5, 6

# --- small f32 table layout -------------------------------------------------
OFF_X0 = 0             # 132: initial [mu | pad | y | pad]
OFF_V = 132            # 65
OFF_GG = 198           # 132: [G1-G2 | G2] aligned to X layout
OFF_MUU0 = 330         # 65: initial MUU (= beta column at l=0)
OFF_A3C0 = 396
OFF_U1A1C0 = 397       # U[1]*A1[0]  (must stay adjacent after A3C0)
OFF_A1C0 = 398
OFF_SIG0 = 399
OFF_MUS = 400          # 16
OFF_LV = 416           # 16
TBL_W = 432

XW = 132   # state: mu 0..64, pad, y 66..130, pad
YO = 66

_CACHED = {}


def _host_tables(batch_input, transition_probs, emission_probs, mus, logvars):
    import ml_dtypes

    a = np.asarray(transition_probs, np.float64)
    Earr = np.exp(np.asarray(emission_probs, np.float64))
    s = np.asarray(batch_input)
    A1 = np.exp(a[:, :, M2M])
    A2 = np.exp(a[:, :, I2M])
    A3 = np.exp(a[:, :, D2M])
    B1 = 0.25 * np.exp(a[:, :, M2I])
    B2 = 0.25 * np.exp(a[:, :, I2I])
    C1 = np.exp(a[:, :, M2D])
    C2 = np.exp(a[:, :, D2D])

    U = np.zeros((B, K1)); V = np.zeros((B, K1))
    U[:, 1:] = A3[:, 1:] * C1[:, :-1] / A1[:, :-1]
    V[:, 1:] = A3[:, 1:] * C2[:, :-1] / A3[:, :-1]
    G1 = A2 * B1 / A1
    G2 = B2

    Etil = A1[:, 1:, None] * Earr
    ee = Etil[np.arange(B)[:, None, None], np.arange(K)[None, None, :],
              s[:, :, None]]                      # (B, L, K)
    ee_bf = np.asarray(ee, ml_dtypes.bfloat16).reshape(B, L * K)
    eeU = np.zeros((B, L, K))
    eeU[:, :, :K - 1] = U[:, None, 2:] * ee[:, :, :K - 1]
    eeU_bf = np.asarray(eeU, ml_dtypes.bfloat16).reshape(B, L * K)

    sig0 = np.exp(NEG - LOGACC0)
    e0 = np.exp(-LOGACC0)
    mu0 = np.empty((B, K1)); iot0 = np.empty((B, K1))
    mu0[:, 0] = A1[:, 0] * e0
    mu0[:, 1:] = A1[:, 1:] * sig0
    iot0[:, :] = A2 * sig0
    muu0 = np.empty((B, K1))
    muu0[:, 0] = A3[:, 0] * sig0
    muu0[:, 1:] = U[:, 1:] * mu0[:, :-1]

    tbl = np.zeros((B, TBL_W), np.float32)
    tbl[:, OFF_X0:OFF_X0 + K1] = mu0
    tbl[:, OFF_X0 + YO:OFF_X0 + YO + K1] = mu0 + iot0
    tbl[:, OFF_V:OFF_V + K1] = V
    tbl[:, OFF_GG:OFF_GG + K1] = G1 - G2
    tbl[:, OFF_GG + YO:OFF_GG + YO + K1] = G2
    tbl[:, OFF_MUU0:OFF_MUU0 + K1] = muu0
    tbl[:, OFF_A1C0] = A1[:, 0]
    tbl[:, OFF_A3C0] = A3[:, 0]
    tbl[:, OFF_SIG0] = sig0
    tbl[:, OFF_U1A1C0] = U[:, 1] * A1[:, 0]
    tbl[:, OFF_MUS:OFF_MUS + E] = np.asarray(mus, np.float32)
    tbl[:, OFF_LV:OFF_LV + E] = np.asarray(logvars, np.float32)
    return tbl, ee_bf, eeU_bf


def _build_bass():
    import concourse.tile as tile
    from concourse import bacc, mybir
    from contextlib import ExitStack

    f32 = mybir.dt.float32
    bf = mybir.dt.bfloat16
    mult = mybir.AluOpType.mult
    add = mybir.AluOpType.add
    mx_op = mybir.AluOpType.max
    AF = mybir.ActivationFunctionType

    nc = bacc.Bacc("TRN2", target_bir_lowering=False, debug=False,
                   num_devices=N_CORES)
    tbl_d = nc.dram_tensor("tbl", [BPC, TBL_W], f32, kind="ExternalInput").ap()
    ee_d = nc.dram_tensor("ee", [BPC, L * K], bf, kind="ExternalInput").ap()
    eeU_d = nc.dram_tensor("eeU", [BPC, L * K], bf, kind="ExternalInput").ap()
    out_d = nc.dram_tensor("loss", [BPC, 1], f32, kind="ExternalOutput").ap()

    with tile.TileContext(nc) as tc, ExitStack() as ctx:
        ctx.enter_context(nc.allow_low_precision(
            reason="bf16 DP state validated to ~2e-4 per-example on the loss"))
        pool = ctx.enter_context(tc.tile_pool(name="p", bufs=1))

        TBL = pool.tile([BPC, TBL_W], f32, tag="TBL", name="TBL")
        EEt = pool.tile([BPC, L * K], bf, tag="EE", name="EE")
        EUt = pool.tile([BPC, L * K], bf, tag="EU", name="EU")
        nc.scalar.dma_start(TBL[:, :], tbl_d[:, :])
        CW = L * K // 4
        bounds = [0, CW // 4, CW, 2 * CW, 3 * CW, 4 * CW]
        for c in range(len(bounds) - 1):
            lo, hi = bounds[c], bounds[c + 1]
            nc.scalar.dma_start(EEt[:, lo:hi], ee_d[:, lo:hi])
            nc.gpsimd.dma_start(EUt[:, lo:hi], eeU_d[:, lo:hi])

        def tb(off, n):
            return TBL[:, off:off + n]

        v = nc.vector

        # ---- DP loop state --------------------------------------------------
        x_pp = [pool.tile([BPC, XW], bf, tag="x_a", name="x_a"),
                pool.tile([BPC, XW], bf, tag="x_b", name="x_b")]
        MUU = pool.tile([BPC, K1], bf, tag="MUU", name="MUU")
        Vb = pool.tile([BPC, K1], bf, tag="Vb", name="Vb")
        GGb = pool.tile([BPC, XW], bf, tag="GGb", name="GGb")
        delta = pool.tile([BPC, K1], bf, tag="delta", name="delta")
        t = pool.tile([BPC, K], bf, tag="t", name="t")
        r12 = pool.tile([BPC, XW], bf, tag="r12", name="r12")
        ya = pool.tile([BPC, K1 + 1], bf, tag="ya", name="ya")
        sig = pool.tile([BPC, 1], f32, tag="sig", name="sig")
        rmxb = pool.tile([BPC, 1], bf, tag="rmxb", name="rmxb")
        mxt = pool.tile([BPC, 1], f32, tag="mxt", name="mxt")
        NRS = len([l for l in range(L) if (l + 1) % R == 0 and (l + 1) < L])
        rhist = pool.tile([BPC, NRS], f32, tag="rhist", name="rhist")

        v.memset(x_pp[0][:, :], 0.0)
        v.memset(x_pp[1][:, :], 0.0)
        v.memset(r12[:, :], 0.0)
        v.tensor_copy(Vb[:, :], tb(OFF_V, K1))
        v.memset(GGb[:, :], 0.0)
        v.tensor_copy(GGb[:, 0:K1], tb(OFF_GG, K1))
        v.tensor_copy(GGb[:, YO:YO + K1], tb(OFF_GG + YO, K1))
        v.tensor_copy(x_pp[0][:, 0:K1], tb(OFF_X0, K1))
        v.tensor_copy(x_pp[0][:, YO:YO + K1], tb(OFF_X0 + YO, K1))
        v.tensor_copy(MUU[:, :], tb(OFF_MUU0, K1))
        v.tensor_copy(sig[:, :], tb(OFF_SIG0, 1))
        v.tensor_mul(x_pp[1][:, 0:1], tb(OFF_A1C0, 1), sig[:, :])

        g = nc.gpsimd

        def dp_step(l):
            X, Xn = x_pp[l % 2], x_pp[(l + 1) % 2]
            v.tensor_tensor_scan(delta[:, :], Vb[:, :], MUU[:, :], 0.0,
                                 mult, add)
            v.tensor_mul(r12[:, :], GGb[:, :], X[:, :])
            v.tensor_add(t[:, :], X[:, YO:YO + K], delta[:, 0:K])
            v.tensor_add(ya[:, :], r12[:, 0:K1 + 1], r12[:, YO:YO + K1 + 1])
            v.tensor_mul(Xn[:, 1:K1], EEt[:, l * K:(l + 1) * K], t[:, :])
            v.tensor_mul(MUU[:, 2:K1], EUt[:, l * K:l * K + K - 1],
                         t[:, 0:K - 1])
            v.tensor_add(Xn[:, YO:YO + K1 + 1], Xn[:, 0:K1 + 1], ya[:, :])

        def rescale(i, l):
            cur = (l + 1) % 2
            Xc = x_pp[cur]
            x_stale = x_pp[1 - cur]
            rmx = rhist[:, i:i + 1]
            v.tensor_reduce(mxt[:, :], Xc[:, YO:YO + K1], mybir.AxisListType.X,
                            mx_op)
            v.reciprocal(rmxb[:, :], mxt[:, :])
            v.tensor_copy(rmx, rmxb[:, :])
            v.tensor_scalar_mul(Xc[:, :], Xc[:, :], rmx)
            v.tensor_scalar_mul(sig[:, :], sig[:, :], rmx)
            v.tensor_scalar_mul(MUU[:, 0:2], tb(OFF_A3C0, 2), sig[:, :])
            v.tensor_mul(x_stale[:, 0:1], tb(OFF_A1C0, 1), sig[:, :])

        for l in range(L):
            dp_step(l)
            if l == 0:
                v.tensor_mul(x_pp[0][:, 0:1], tb(OFF_A1C0, 1), sig[:, :])
                v.tensor_mul(MUU[:, 1:2], tb(OFF_U1A1C0, 1), sig[:, :])
            if (l + 1) % R == 0 and (l + 1) < L:
                rescale((l + 1) // R - 1, l)

        # KLD (after the loop: keeps act-table loads and these ops off the
        # critical path at kernel start)
        ev = pool.tile([BPC, E], f32, tag="ev", name="ev")
        sq = pool.tile([BPC, E], f32, tag="sq", name="sq")
        w1 = pool.tile([BPC, E], f32, tag="w1", name="w1")
        w2 = pool.tile([BPC, E], f32, tag="w2", name="w2")
        red = pool.tile([BPC, 1], f32, tag="red", name="red")
        kld = pool.tile([BPC, 1], f32, tag="kld", name="kld")
        nc.scalar.activation(ev[:, :], tb(OFF_LV, E), AF.Exp)
        nc.scalar.activation(sq[:, :], tb(OFF_MUS, E), AF.Square)
        v.tensor_sub(w1[:, :], tb(OFF_LV, E), sq[:, :])
        v.tensor_sub(w2[:, :], w1[:, :], ev[:, :])
        v.tensor_reduce(red[:, :], w2[:, :], mybir.AxisListType.X, add)
        v.tensor_scalar(kld[:, :], red[:, :], -0.5, -float(E) / 2.0, mult, add)

        Xf = x_pp[L % 2]
        tf = pool.tile([BPC, K1], f32, tag="tf", name="tf")
        lnp = pool.tile([BPC, 1], f32, tag="lnp", name="lnp")
        lnr = pool.tile([BPC, NRS], f32, tag="lnr", name="lnr")
        sumlr = pool.tile([BPC, 1], f32, tag="sumlr", name="sumlr")
        lacc = pool.tile([BPC, 1], f32, tag="lacc", name="lacc")
        nv = pool.tile([BPC, 1], f32, tag="nv", name="nv")
        v.tensor_tensor_scan(delta[:, :], Vb[:, :], MUU[:, :], 0.0, mult, add)
        v.tensor_add(tf[:, :], Xf[:, YO:YO + K1], delta[:, :])
        nc.scalar.activation(lnp[:, :], tf[:, K:K1], AF.Ln)
        nc.scalar.activation(lnr[:, :], rhist[:, :], AF.Ln)
        v.tensor_reduce(sumlr[:, :], lnr[:, :], mybir.AxisListType.X, add)
        v.tensor_scalar(lacc[:, :], sumlr[:, :], -1.0, LOGACC0, mult, add)
        v.tensor_add(nv[:, :], lnp[:, :], lacc[:, :])
        loss_t = pool.tile([BPC, 1], f32, tag="loss_t", name="loss_t")
        v.tensor_sub(loss_t[:, :], kld[:, :], nv[:, :])
        nc.sync.dma_start(out_d[:, :], loss_t[:, :])

    nc.compile()
    return nc


def _get_nc():
    if "nc" not in _CACHED:
        _CACHED["nc"] = _build_bass()
    return _CACHED["nc"]


def kernel(batch_input, transition_probs, emission_probs, mus, logvars):
    from concourse.bass_utils import run_bass_kernel_spmd

    tbl, ee, eeU = _host_tables(batch_input, transition_probs, emission_probs,
                                mus, logvars)
    nc = _get_nc()
    in_maps = [{"tbl": tbl[c * BPC:(c + 1) * BPC],
                "ee": ee[c * BPC:(c + 1) * BPC],
                "eeU": eeU[c * BPC:(c + 1) * BPC]} for c in range(N_CORES)]
    res = run_bass_kernel_spmd(nc, in_maps, list(range(N_CORES)))
    losses = np.concatenate([np.asarray(r["loss"])[:, 0] for r in res.results])
    return np.float32(np.mean(losses.astype(np.float64)))
